# revision 17
# baseline (speedup 1.0000x reference)
"""CTC loss (nn_CTCLoss) Trainium2 Bass kernel, v4.

Sharding: data-parallel over batch N across 8 cores (8 samples/core).

Per core, two decoupled device pipelines:
  * DP feed: the host pre-gathers the S=2L+1 extended-label logits per
    (chain, sample, step) into a slab qx [16, (T/2)*S] bf16 (row p<8 =
    forward chain of sample p, row p>=8 = backward chain, time- and
    state-reversed so both chains read forward).  States are stored
    de-interleaved per step: [labels (L), blanks (L+1)], which turns the
    CTC three-tap recurrence into contiguous short ops.  Two early DMAs
    (first 16-step block, then the rest) bring it in; ScalarE
    exponentiates per 16-step block (q = exp(x + SHIFT), f32 out);
    VectorE runs the UNNORMALISED DP in the probability domain:
        u  = B + shift(Lb)        [L+1 wide]   (blank update pre-mul)
        w  = u + Lb               [L wide]     (label update pre-mul)
        A' = [w | u] * q          [S wide]
    i.e. 3 short ops per time step, T/2 sequential steps, both chains
    advancing together on disjoint partitions.  No per-step softmax
    normalisation: a constant SHIFT keeps the chain inside f32 range and
    the true denominators are restored on host.
  * Denominators: the (T, NL, C) shard streams through SBUF as 16
    [128, C] f32 tiles (one 2 MB contiguous DMA each); ScalarE computes
    exp() with a fused per-row accumulate, collecting the softmax
    denominator of every (t, n) into accT [128, 16].

Final alpha/beta states plus accT go back to the host, which stitches
the chains at the midpoint in float64 and applies sum(log acc) +
T*SHIFT before the batch mean.
"""

import sys

import numpy as np

for _p in ("/root/.axon_site", "/root/.axon_site/_ro/trn_rl_repo", "/opt/trn_rl_repo"):
    if _p not in sys.path:
        sys.path.append(_p)

NCORES = 8
NL = 8                   # samples per core
TG = 16                  # time steps per [128, C] tile and per q block
BLANK = 0
PAD = 2                  # leading zero pad columns in DP tiles

# problem dims (the graded configuration)
T, N, C, L = 256, 64, 4096, 32

SHIFT2 = -1.0            # constant per-step scale: q = exp(x + SHIFT2)


def _derived(T_, C_, L_):
    S_ = 2 * L_ + 1
    NG_ = T_ // TG                     # [128, C] tile groups
    TH_ = T_ // 2                      # steps per chain
    NB_ = TH_ // TG                    # q blocks
    return S_, NG_, TH_, NB_


# ----------------------------------------------------------------------------
# host-side helpers
# ----------------------------------------------------------------------------

def _ext_labels(t2d, S_):
    ext = np.zeros((t2d.shape[0], S_), np.int64)
    ext[:, 1::2] = t2d
    return ext


def _skip_mask(ext):
    sidx = np.arange(ext.shape[1])
    return (
        (sidx[None, :] >= 2)
        & (ext != BLANK)
        & (ext != np.roll(ext, 2, axis=1))
    )


def _ref_numpy(preds, t2d, pred_lengths, target_lengths):
    """float64 port of the reference (fallback path)."""
    preds = preds.astype(np.float64)
    Tn, n = preds.shape[0], preds.shape[1]
    S_ = 2 * t2d.shape[1] + 1
    mx = preds.max(axis=2, keepdims=True)
    lp = preds - mx - np.log(np.exp(preds - mx).sum(axis=2, keepdims=True))
    ext = _ext_labels(t2d, S_)
    lpe = lp[:, np.arange(n)[:, None], ext]
    skip_ok = _skip_mask(ext)
    NEGI = -1e30
    sidx = np.arange(S_)
    valid = sidx[None, :] < (2 * target_lengths[:, None] + 1)
    alpha = np.full((n, S_), NEGI)
    alpha[:, 0] = lpe[0, :, 0]
    alpha[:, 1] = np.where(target_lengths > 0, lpe[0, :, 1], NEGI)
    alpha = np.where(valid, alpha, NEGI)

    def lse(*a):
        m = np.maximum.reduce(a)
        m = np.where(np.isfinite(m), m, 0.0)
        return m + np.log(sum(np.exp(x - m) for x in a))

    for t in range(1, Tn):
        a2 = np.concatenate([np.full((n, 1), NEGI), alpha[:, :-1]], 1)
        a3 = np.concatenate([np.full((n, 2), NEGI), alpha[:, :-2]], 1)
        a3 = np.where(skip_ok, a3, NEGI)
        new = np.where(valid, lse(alpha, a2, a3) + lpe[t], NEGI)
        active = (t < pred_lengths)[:, None]
        alpha = np.where(active, new, alpha)
    end = 2 * target_lengths
    a_last = alpha[np.arange(n), end]
    a_prev = alpha[np.arange(n), np.maximum(end - 1, 0)]
    a_prev = np.where(target_lengths > 0, a_prev, NEGI)
    nll = -lse(a_last, a_prev)
    nll = np.where(np.isfinite(nll) & (nll < 1e29), nll, 0.0)
    return np.float32(np.mean(nll / np.maximum(target_lengths, 1)))


# ----------------------------------------------------------------------------
# kernel builder
# ----------------------------------------------------------------------------

_NC_CACHE = {}


def _build(use_masks, dims):
    """use_masks=False: de-interleaved [Lb | B] DP layout (no repeated
    labels).  use_masks=True: natural state order with mask tiles."""
    T_, C_, L_ = dims
    S_, NG_, TH_, NB_ = _derived(T_, C_, L_)
    PL = PAD + L_            # start of the blank block (no-repeat layout)

    import concourse.bacc as bacc
    import concourse.tile as tile
    from concourse import mybir

    f32 = mybir.dt.float32
    bf16 = mybir.dt.bfloat16
    Act = mybir.ActivationFunctionType

    nc = bacc.Bacc("TRN2", target_bir_lowering=False, debug=False)
    # preds shard pre-tiled on host: [group pair, n, t16, 2*c] (two tile
    # groups side by side per row); each [128, 2C] tile load reads 128
    # consecutive 32KB rows (full HBM bandwidth, one wait per 4MB)
    px = nc.dram_tensor("px", [NG_ // 2, NL, TG, 2 * C_], f32,
                        kind="ExternalInput")
    # host-gathered extended-label logits in DP order (see module docstring)
    qx = nc.dram_tensor("qx", [16, TH_ * S_], bf16, kind="ExternalInput")
    if use_masks:
        maskd = nc.dram_tensor("maskd", [16, PAD + S_], f32,
                               kind="ExternalInput")
    res = nc.dram_tensor("res", [16, PAD + S_], f32, kind="ExternalOutput")
    accd = nc.dram_tensor("accd", [128, NG_], f32, kind="ExternalOutput")

    with tile.TileContext(nc) as tc:
        with (
            tc.tile_pool(name="mt", bufs=3) as mt_pool,
            tc.tile_pool(name="scr", bufs=2) as scr_pool,
            tc.tile_pool(name="single", bufs=1) as single,
        ):
            with tc.high_priority():
                # DP feed: issued first on the sync queue so its ring
                # descriptors precede the 2MB tile flood
                qxt0 = single.tile([16, TG * S_], bf16, tag="qxt0")
                nc.sync.dma_start(out=qxt0[:], in_=qx[:, 0:TG * S_])
                qxtr = single.tile([16, (TH_ - TG) * S_], bf16, tag="qxtr")
                nc.sync.dma_start(out=qxtr[:], in_=qx[:, TG * S_:])
                if use_masks:
                    msk = single.tile([16, PAD + S_], f32, tag="msk")
                    nc.sync.dma_start(out=msk[:], in_=maskd[:])

                shiftb = single.tile([16, 1], f32, tag="shiftb")
                nc.vector.memset(shiftb[:], SHIFT2)
                # dummy activation: pulls ACT_TABLE_LOAD to the very start
                warm = single.tile([16, 1], f32, tag="warm")
                nc.scalar.activation(warm[:], shiftb[:], Act.Exp,
                                     bias=0.0, scale=1.0)

                A = single.tile([16, PAD + S_], f32, tag="A")
                t1 = single.tile([16, PAD + S_], f32, tag="t1")
                nc.vector.memset(A[:], 0.0)
                nc.vector.memset(t1[:], 0.0)
                if use_masks:
                    am = single.tile([16, PAD + S_], f32, tag="am")
                    nc.vector.memset(am[:], 0.0)

                # q blocks: exp of the host-gathered slab
                qct = []
                for j in range(NB_):
                    qcj = single.tile([16, TG * S_], f32, tag=f"qc{j}",
                                      name=f"qc_{j}")
                    if j == 0:
                        src = qxt0[:]
                    else:
                        src = qxtr[:, (j - 1) * TG * S_:j * TG * S_]
                    nc.scalar.activation(qcj[:], src, Act.Exp,
                                         bias=shiftb[:, 0:1], scale=1.0)
                    qct.append(qcj)

            # two accumulator tiles so consecutive big exps have no WAW chain
            accTa = single.tile([128, NG_ // 2], f32, tag="accTa")
            accTb = single.tile([128, NG_ // 2], f32, tag="accTb")

            # softmax denominators: stream the shard, fused exp+row-sum;
            # two activations per 4MB tile -> one DMA wait per pair
            for j in range(NG_ // 2):
                mt = mt_pool.tile([128, 2 * C_], f32, tag="mt")
                nc.scalar.dma_start(out=mt[:],
                                    in_=px[j].rearrange("n t c -> (n t) c"))
                scr = scr_pool.tile([128, C_], bf16, tag="scr")
                nc.scalar.activation(scr[:], mt[:, 0:C_], Act.Exp,
                                     bias=0.0, scale=1.0,
                                     accum_out=accTa[:, j:j + 1])
                scr2 = scr_pool.tile([128, C_], bf16, tag="scr")
                nc.scalar.activation(scr2[:], mt[:, C_:2 * C_], Act.Exp,
                                     bias=0.0, scale=1.0,
                                     accum_out=accTb[:, j:j + 1])

            # DP: T/2 sequential steps, fwd chain on partitions 0-7 and
            # (time/state-reversed) bwd chain on 8-15 advancing together
            for k in range(TH_):
                qc = qct[k // TG]
                o = (k % TG) * S_
                qk = qc[:, o:o + S_]
                if k == 0:
                    if use_masks:
                        nc.vector.tensor_copy(A[:, PAD:PAD + 2], qc[:, 0:2])
                        nc.vector.tensor_mul(am[:, PAD:], A[:, PAD:],
                                             msk[:, PAD:])
                    else:
                        # A[Lb 0] = q[l0], A[B 0] = q[b0] (cols o, o+L)
                        dst = A[:, PAD:PAD + 2 * L_].rearrange(
                            "p (a b) -> p a b", b=L_)[:, :, 0]
                        src = qc[:, o:o + 2 * L_].rearrange(
                            "p (a b) -> p a b", b=L_)[:, :, 0]
                        nc.vector.tensor_copy(dst, src)
                    continue
                if use_masks:
                    # t1 = A + shift1(A); t1 += shift2(masked A); A' = t1*q
                    nc.vector.tensor_add(t1[:, PAD:], A[:, PAD:],
                                         A[:, PAD - 1:PAD + S_ - 1])
                    nc.vector.tensor_add(t1[:, PAD:], t1[:, PAD:],
                                         am[:, 0:S_])
                    nc.vector.tensor_mul(A[:, PAD:], t1[:, PAD:], qk)
                    nc.vector.tensor_mul(am[:, PAD:], A[:, PAD:],
                                         msk[:, PAD:])
                else:
                    # u = B + shift(Lb)  (blank pre-mul, L+1 wide)
                    nc.vector.tensor_add(t1[:, PL:PL + L_ + 1],
                                         A[:, PL:PL + L_ + 1],
                                         A[:, PAD - 1:PAD + L_])
                    # w = u + Lb  (label pre-mul, L wide)
                    nc.vector.tensor_add(t1[:, PAD:PAD + L_],
                                         t1[:, PL:PL + L_],
                                         A[:, PAD:PAD + L_])
                    # A' = [w | u] * q
                    nc.vector.tensor_mul(A[:, PAD:PAD + S_],
                                         t1[:, PAD:PAD + S_], qk)

            nc.scalar.dma_start(out=accd[:, 0:NG_ // 2], in_=accTa[:])
            nc.scalar.dma_start(out=accd[:, NG_ // 2:NG_], in_=accTb[:])
            nc.sync.dma_start(out=res[:], in_=A[:])
    nc.compile()
    return nc


def _get_nc(use_masks, dims):
    key = (use_masks, dims)
    if key not in _NC_CACHE:
        _NC_CACHE[key] = _build(use_masks, dims)
    return _NC_CACHE[key]


# ----------------------------------------------------------------------------
# device run for one full (T_, N=64, C_) problem
# ----------------------------------------------------------------------------

def _run_device(preds, t2d, dims):
    T_, C_, L_ = dims
    S_, NG_, TH_, NB_ = _derived(T_, C_, L_)
    import ml_dtypes

    ext = _ext_labels(t2d, S_)                    # (N, S)
    m_fwd = _skip_mask(ext)
    use_masks = bool((t2d[:, 1:] == t2d[:, :-1]).any())

    # m'[s] = m[s+2] (allowed s -> s+2); backward chain is state-reversed
    m_p = np.zeros_like(m_fwd)
    m_p[:, :-2] = m_fwd[:, 2:]
    m_bwd = m_p[:, ::-1]

    from concourse.bass_utils import run_bass_kernel_spmd

    nc = _get_nc(use_masks, dims)

    # gather column orders: natural (masks) or [labels | blanks] split
    if use_masks:
        cols_f = ext                               # (N, S)
        cols_b = ext[:, ::-1]
    else:
        cols_f = np.concatenate([ext[:, 1::2], ext[:, 0::2]], axis=1)
        extr = ext[:, ::-1]
        cols_b = np.concatenate([extr[:, 1::2], extr[:, 0::2]], axis=1)

    in_maps = []
    for c in range(NCORES):
        n0 = c * NL
        sh = preds[:, n0:n0 + NL, :]               # (T, NL, C)
        # pre-tile: (T, NL, C) -> (NG/2, NL, TG, 2C): group pairs side by
        # side along the free axis, (n, t16) row order
        t4 = sh.reshape(NG_, TG, NL, C_).transpose(0, 2, 1, 3)
        tiles = np.ascontiguousarray(
            np.concatenate([t4[0::2], t4[1::2]], axis=3))
        # host-gathered DP slab [16, TH*S]
        nidx = np.arange(NL)[:, None, None]
        kidx = np.arange(TH_)[None, :, None]
        qf = sh[kidx, nidx, cols_f[n0:n0 + NL, None, :]]         # (NL,TH,S)
        qb = sh[T_ - 1 - kidx, nidx, cols_b[n0:n0 + NL, None, :]]
        qxa = np.empty((16, TH_ * S_), np.float32)
        qxa[0:NL] = qf.reshape(NL, TH_ * S_)
        qxa[NL:16] = qb.reshape(NL, TH_ * S_)
        im = {"px": tiles, "qx": qxa.astype(ml_dtypes.bfloat16)}
        if use_masks:
            # am-premask: am[x] = A[x] * M[x+2] so that am[s-2] carries the
            # destination mask M[s]
            mam_f = np.zeros_like(m_fwd)
            mam_f[:, :-2] = m_fwd[:, 2:]
            mam_b = np.zeros_like(m_bwd)
            mam_b[:, :-2] = m_bwd[:, 2:]
            mtile = np.zeros((16, PAD + S_), np.float32)
            mtile[0:NL, PAD:] = mam_f[n0:n0 + NL]
            mtile[NL:16, PAD:] = mam_b[n0:n0 + NL]
            im["maskd"] = mtile
        in_maps.append(im)

    out = run_bass_kernel_spmd(nc, in_maps, core_ids=list(range(NCORES)))

    # host stitch (float64): combine the two chains at the midpoint and
    # restore the softmax denominators: ll = ln v - sum_t ln acc - T*SHIFT
    losses = np.zeros(NCORES * NL, np.float64)
    for c in range(NCORES):
        resv = np.asarray(out.results[c]["res"]).astype(np.float64)
        accv = np.asarray(out.results[c]["accd"]).astype(np.float64)
        if not (np.isfinite(accv).all() and (accv > 0).all()):
            return None
        lacc = np.log(accv).reshape(NL, TG, NG_).sum(axis=(1, 2))  # per n
        for n in range(NL):
            gn = c * NL + n
            ar = resv[n, PAD:]            # alpha_{TH-1}
            br = resv[NL + n, PAD:]       # beta_{TH}, reversed s order
            if use_masks:
                a, b = ar, br
            else:
                # de-interleave [labels | blanks] back to natural order
                a = np.empty(S_)
                a[1::2] = ar[0:L_]
                a[0::2] = ar[L_:S_]
                b = np.empty(S_)
                b[1::2] = br[0:L_]
                b[0::2] = br[L_:S_]
            mb = m_bwd[gn]
            be = b.copy()
            be[1:] += b[:-1]
            be[2:] += np.where(mb[2:], b[:-2], 0.0)
            v = float((a[::-1] * be).sum())
            if not (np.isfinite(v) and v > 0.0):
                return None
            ll = np.log(v) - lacc[n] - T_ * SHIFT2
            losses[gn] = -ll / L_
    return np.float32(losses.mean())


# ----------------------------------------------------------------------------
# entry point
# ----------------------------------------------------------------------------

def kernel(preds, targets, pred_lengths, target_lengths):
    preds = np.asarray(preds, np.float32)
    targets = np.asarray(targets, np.int32)
    pred_lengths = np.asarray(pred_lengths, np.int32)
    target_lengths = np.asarray(target_lengths, np.int32)
    t2d = targets.reshape(N, L)

    fast_ok = (
        preds.shape == (T, N, C)
        and targets.shape == (N * L,)
        and np.all(pred_lengths == T)
        and np.all(target_lengths == L)
        and np.all(targets >= 1)
        and np.all(targets < C)
        and np.isfinite(preds).all()
        and np.abs(preds).max() < 8.0
    )
    if fast_ok:
        r = _run_device(preds, t2d, (T, C, L))
        if r is not None:
            return r
    return _ref_numpy(preds, t2d, pred_lengths, target_lengths)


# revision 20
# speedup vs baseline: 1.1072x; 1.1072x over previous
"""CTC loss (nn_CTCLoss) Trainium2 Bass kernel, v4.

Sharding: data-parallel over batch N across 8 cores (8 samples/core).

Per core, two decoupled device pipelines:
  * DP feed: the host pre-gathers the S=2L+1 extended-label logits per
    (chain, sample, step) into a slab qx [16, (T/2)*S] bf16 (row p<8 =
    forward chain of sample p, row p>=8 = backward chain, time- and
    state-reversed so both chains read forward).  States are stored
    de-interleaved per step: [labels (L), blanks (L+1)], which turns the
    CTC three-tap recurrence into contiguous short ops.  Two early DMAs
    (first 16-step block, then the rest) bring it in; ScalarE
    exponentiates per 16-step block (q = exp(x + SHIFT), f32 out);
    VectorE runs the UNNORMALISED DP in the probability domain:
        u  = B + shift(Lb)        [L+1 wide]   (blank update pre-mul)
        w  = u + Lb               [L wide]     (label update pre-mul)
        A' = [w | u] * q          [S wide]
    i.e. 3 short ops per time step, T/2 sequential steps, both chains
    advancing together on disjoint partitions.  No per-step softmax
    normalisation: a constant SHIFT keeps the chain inside f32 range and
    the true denominators are restored on host.
  * Denominators: the (T, NL, C) shard streams through SBUF as 16
    [128, C] f32 tiles (one 2 MB contiguous DMA each); ScalarE computes
    exp() with a fused per-row accumulate, collecting the softmax
    denominator of every (t, n) into accT [128, 16].

Final alpha/beta states plus accT go back to the host, which stitches
the chains at the midpoint in float64 and applies sum(log acc) +
T*SHIFT before the batch mean.
"""

import sys

import numpy as np

for _p in ("/root/.axon_site", "/root/.axon_site/_ro/trn_rl_repo", "/opt/trn_rl_repo"):
    if _p not in sys.path:
        sys.path.append(_p)

NCORES = 8
NL = 8                   # samples per core
TG = 16                  # time steps per [128, C] tile and per q block
BLANK = 0
PAD = 2                  # leading zero pad columns in DP tiles

# problem dims (the graded configuration)
T, N, C, L = 256, 64, 4096, 32

SHIFT2 = -1.0            # constant per-step scale: q = exp(x + SHIFT2)


def _derived(T_, C_, L_):
    S_ = 2 * L_ + 1
    NG_ = T_ // TG                     # [128, C] tile groups
    TH_ = T_ // 2                      # steps per chain
    NB_ = TH_ // TG                    # q blocks
    return S_, NG_, TH_, NB_


# ----------------------------------------------------------------------------
# host-side helpers
# ----------------------------------------------------------------------------

def _ext_labels(t2d, S_):
    ext = np.zeros((t2d.shape[0], S_), np.int64)
    ext[:, 1::2] = t2d
    return ext


def _skip_mask(ext):
    sidx = np.arange(ext.shape[1])
    return (
        (sidx[None, :] >= 2)
        & (ext != BLANK)
        & (ext != np.roll(ext, 2, axis=1))
    )


def _ref_numpy(preds, t2d, pred_lengths, target_lengths):
    """float64 port of the reference (fallback path)."""
    preds = preds.astype(np.float64)
    Tn, n = preds.shape[0], preds.shape[1]
    S_ = 2 * t2d.shape[1] + 1
    mx = preds.max(axis=2, keepdims=True)
    lp = preds - mx - np.log(np.exp(preds - mx).sum(axis=2, keepdims=True))
    ext = _ext_labels(t2d, S_)
    lpe = lp[:, np.arange(n)[:, None], ext]
    skip_ok = _skip_mask(ext)
    NEGI = -1e30
    sidx = np.arange(S_)
    valid = sidx[None, :] < (2 * target_lengths[:, None] + 1)
    alpha = np.full((n, S_), NEGI)
    alpha[:, 0] = lpe[0, :, 0]
    alpha[:, 1] = np.where(target_lengths > 0, lpe[0, :, 1], NEGI)
    alpha = np.where(valid, alpha, NEGI)

    def lse(*a):
        m = np.maximum.reduce(a)
        m = np.where(np.isfinite(m), m, 0.0)
        return m + np.log(sum(np.exp(x - m) for x in a))

    for t in range(1, Tn):
        a2 = np.concatenate([np.full((n, 1), NEGI), alpha[:, :-1]], 1)
        a3 = np.concatenate([np.full((n, 2), NEGI), alpha[:, :-2]], 1)
        a3 = np.where(skip_ok, a3, NEGI)
        new = np.where(valid, lse(alpha, a2, a3) + lpe[t], NEGI)
        active = (t < pred_lengths)[:, None]
        alpha = np.where(active, new, alpha)
    end = 2 * target_lengths
    a_last = alpha[np.arange(n), end]
    a_prev = alpha[np.arange(n), np.maximum(end - 1, 0)]
    a_prev = np.where(target_lengths > 0, a_prev, NEGI)
    nll = -lse(a_last, a_prev)
    nll = np.where(np.isfinite(nll) & (nll < 1e29), nll, 0.0)
    return np.float32(np.mean(nll / np.maximum(target_lengths, 1)))


# ----------------------------------------------------------------------------
# kernel builder
# ----------------------------------------------------------------------------

_NC_CACHE = {}


def _build(use_masks, dims):
    """use_masks=False: de-interleaved [Lb | B] DP layout (no repeated
    labels).  use_masks=True: natural state order with mask tiles."""
    T_, C_, L_ = dims
    S_, NG_, TH_, NB_ = _derived(T_, C_, L_)
    PL = PAD + L_            # start of the blank block (no-repeat layout)

    import concourse.bacc as bacc
    import concourse.tile as tile
    from concourse import mybir

    f32 = mybir.dt.float32
    bf16 = mybir.dt.bfloat16
    Act = mybir.ActivationFunctionType

    nc = bacc.Bacc("TRN2", target_bir_lowering=False, debug=False)
    # preds shard pre-tiled on host: [group pair, n, t16, 2*c] (two tile
    # groups side by side per row); each [128, 2C] tile load reads 128
    # consecutive 32KB rows (full HBM bandwidth, one wait per 4MB)
    px = nc.dram_tensor("px", [NG_ // 2, NL, TG, 2 * C_], f32,
                        kind="ExternalInput")
    # host-gathered extended-label logits in DP order (see module docstring)
    qx = nc.dram_tensor("qx", [16, TH_ * S_], bf16, kind="ExternalInput")
    if use_masks:
        maskd = nc.dram_tensor("maskd", [16, PAD + S_], f32,
                               kind="ExternalInput")
    res = nc.dram_tensor("res", [16, PAD + S_], f32, kind="ExternalOutput")
    accd = nc.dram_tensor("accd", [128, NG_], f32, kind="ExternalOutput")

    with tile.TileContext(nc) as tc:
        with (
            tc.tile_pool(name="mt", bufs=4) as mt_pool,
            tc.tile_pool(name="scr", bufs=2) as scr_pool,
            tc.tile_pool(name="qc", bufs=4) as qc_pool,
            tc.tile_pool(name="single", bufs=1) as single,
        ):
            with tc.high_priority():
                # DP feed: issued first on the sync queue so its ring
                # descriptors precede the 2MB tile flood
                qxt0 = single.tile([16, TG * S_], bf16, tag="qxt0")
                nc.sync.dma_start(out=qxt0[:], in_=qx[:, 0:TG * S_])
                qxtr = single.tile([16, (TH_ - TG) * S_], bf16, tag="qxtr")
                nc.sync.dma_start(out=qxtr[:], in_=qx[:, TG * S_:])
                if use_masks:
                    msk = single.tile([16, PAD + S_], f32, tag="msk")
                    nc.sync.dma_start(out=msk[:], in_=maskd[:])

                shiftb = single.tile([16, 1], f32, tag="shiftb")
                nc.vector.memset(shiftb[:], SHIFT2)
                # dummy activation: pulls ACT_TABLE_LOAD to the very start
                warm = single.tile([16, 1], f32, tag="warm")
                nc.scalar.activation(warm[:], shiftb[:], Act.Exp,
                                     bias=0.0, scale=1.0)

                A = single.tile([16, PAD + S_], f32, tag="A")
                t1 = single.tile([16, PAD + S_], f32, tag="t1")
                nc.vector.memset(A[:], 0.0)
                nc.vector.memset(t1[:], 0.0)
                if use_masks:
                    am = single.tile([16, PAD + S_], f32, tag="am")
                    nc.vector.memset(am[:], 0.0)

                # q blocks: exp of the host-gathered slab (rotating pool;
                # the DP frees a block as it advances past it)
                qct = []
                for j in range(NB_):
                    qcj = qc_pool.tile([16, TG * S_], f32, tag="qc",
                                       name=f"qc_{j}")
                    if j == 0:
                        src = qxt0[:]
                    else:
                        src = qxtr[:, (j - 1) * TG * S_:j * TG * S_]
                    nc.scalar.activation(qcj[:], src, Act.Exp,
                                         bias=shiftb[:, 0:1], scale=1.0)
                    qct.append(qcj)

            # two accumulator tiles so consecutive big exps have no WAW chain
            accTa = single.tile([128, NG_ // 2], f32, tag="accTa")
            accTb = single.tile([128, NG_ // 2], f32, tag="accTb")

            # softmax denominators: stream the shard, fused exp+row-sum;
            # two activations per 4MB tile -> one DMA wait per pair
            for j in range(NG_ // 2):
                mt = mt_pool.tile([128, 2 * C_], f32, tag="mt")
                nc.sync.dma_start(out=mt[:],
                                  in_=px[j].rearrange("n t c -> (n t) c"))
                scr = scr_pool.tile([128, C_], bf16, tag="scr")
                nc.scalar.activation(scr[:], mt[:, 0:C_], Act.Exp,
                                     bias=0.0, scale=1.0,
                                     accum_out=accTa[:, j:j + 1])
                scr2 = scr_pool.tile([128, C_], bf16, tag="scr")
                nc.scalar.activation(scr2[:], mt[:, C_:2 * C_], Act.Exp,
                                     bias=0.0, scale=1.0,
                                     accum_out=accTb[:, j:j + 1])

            # DP: T/2 sequential steps, fwd chain on partitions 0-7 and
            # (time/state-reversed) bwd chain on 8-15 advancing together
            for k in range(TH_):
                qc = qct[k // TG]
                o = (k % TG) * S_
                qk = qc[:, o:o + S_]
                if k == 0:
                    if use_masks:
                        nc.vector.tensor_copy(A[:, PAD:PAD + 2], qc[:, 0:2])
                        nc.vector.tensor_mul(am[:, PAD:], A[:, PAD:],
                                             msk[:, PAD:])
                    else:
                        # A[Lb 0] = q[l0], A[B 0] = q[b0] (cols o, o+L)
                        dst = A[:, PAD:PAD + 2 * L_].rearrange(
                            "p (a b) -> p a b", b=L_)[:, :, 0]
                        src = qc[:, o:o + 2 * L_].rearrange(
                            "p (a b) -> p a b", b=L_)[:, :, 0]
                        nc.vector.tensor_copy(dst, src)
                    continue
                if use_masks:
                    # t1 = A + shift1(A); t1 += shift2(masked A); A' = t1*q
                    nc.vector.tensor_add(t1[:, PAD:], A[:, PAD:],
                                         A[:, PAD - 1:PAD + S_ - 1])
                    nc.vector.tensor_add(t1[:, PAD:], t1[:, PAD:],
                                         am[:, 0:S_])
                    nc.vector.tensor_mul(A[:, PAD:], t1[:, PAD:], qk)
                    nc.vector.tensor_mul(am[:, PAD:], A[:, PAD:],
                                         msk[:, PAD:])
                else:
                    # u = B + shift(Lb)  (blank pre-mul, L+1 wide)
                    nc.vector.tensor_add(t1[:, PL:PL + L_ + 1],
                                         A[:, PL:PL + L_ + 1],
                                         A[:, PAD - 1:PAD + L_])
                    # w = u + Lb  (label pre-mul, L wide)
                    nc.vector.tensor_add(t1[:, PAD:PAD + L_],
                                         t1[:, PL:PL + L_],
                                         A[:, PAD:PAD + L_])
                    # A' = [w | u] * q
                    nc.vector.tensor_mul(A[:, PAD:PAD + S_],
                                         t1[:, PAD:PAD + S_], qk)

            nc.scalar.dma_start(out=accd[:, 0:NG_ // 2], in_=accTa[:])
            nc.scalar.dma_start(out=accd[:, NG_ // 2:NG_], in_=accTb[:])
            nc.sync.dma_start(out=res[:], in_=A[:])
    nc.compile()
    return nc


def _get_nc(use_masks, dims):
    key = (use_masks, dims)
    if key not in _NC_CACHE:
        _NC_CACHE[key] = _build(use_masks, dims)
    return _NC_CACHE[key]


# ----------------------------------------------------------------------------
# device run for one full (T_, N=64, C_) problem
# ----------------------------------------------------------------------------

def _run_device(preds, t2d, dims):
    T_, C_, L_ = dims
    S_, NG_, TH_, NB_ = _derived(T_, C_, L_)
    import ml_dtypes

    ext = _ext_labels(t2d, S_)                    # (N, S)
    m_fwd = _skip_mask(ext)
    use_masks = bool((t2d[:, 1:] == t2d[:, :-1]).any())

    # m'[s] = m[s+2] (allowed s -> s+2); backward chain is state-reversed
    m_p = np.zeros_like(m_fwd)
    m_p[:, :-2] = m_fwd[:, 2:]
    m_bwd = m_p[:, ::-1]

    from concourse.bass_utils import run_bass_kernel_spmd

    nc = _get_nc(use_masks, dims)

    # gather column orders: natural (masks) or [labels | blanks] split
    if use_masks:
        cols_f = ext                               # (N, S)
        cols_b = ext[:, ::-1]
    else:
        cols_f = np.concatenate([ext[:, 1::2], ext[:, 0::2]], axis=1)
        extr = ext[:, ::-1]
        cols_b = np.concatenate([extr[:, 1::2], extr[:, 0::2]], axis=1)

    in_maps = []
    for c in range(NCORES):
        n0 = c * NL
        sh = preds[:, n0:n0 + NL, :]               # (T, NL, C)
        # pre-tile: (T, NL, C) -> (NG/2, NL, TG, 2C): group pairs side by
        # side along the free axis, (n, t16) row order
        t4 = sh.reshape(NG_, TG, NL, C_).transpose(0, 2, 1, 3)
        tiles = np.ascontiguousarray(
            np.concatenate([t4[0::2], t4[1::2]], axis=3))
        # host-gathered DP slab [16, TH*S]
        nidx = np.arange(NL)[:, None, None]
        kidx = np.arange(TH_)[None, :, None]
        qf = sh[kidx, nidx, cols_f[n0:n0 + NL, None, :]]         # (NL,TH,S)
        qb = sh[T_ - 1 - kidx, nidx, cols_b[n0:n0 + NL, None, :]]
        qxa = np.empty((16, TH_ * S_), np.float32)
        qxa[0:NL] = qf.reshape(NL, TH_ * S_)
        qxa[NL:16] = qb.reshape(NL, TH_ * S_)
        im = {"px": tiles, "qx": qxa.astype(ml_dtypes.bfloat16)}
        if use_masks:
            # am-premask: am[x] = A[x] * M[x+2] so that am[s-2] carries the
            # destination mask M[s]
            mam_f = np.zeros_like(m_fwd)
            mam_f[:, :-2] = m_fwd[:, 2:]
            mam_b = np.zeros_like(m_bwd)
            mam_b[:, :-2] = m_bwd[:, 2:]
            mtile = np.zeros((16, PAD + S_), np.float32)
            mtile[0:NL, PAD:] = mam_f[n0:n0 + NL]
            mtile[NL:16, PAD:] = mam_b[n0:n0 + NL]
            im["maskd"] = mtile
        in_maps.append(im)

    out = run_bass_kernel_spmd(nc, in_maps, core_ids=list(range(NCORES)))

    # host stitch (float64): combine the two chains at the midpoint and
    # restore the softmax denominators: ll = ln v - sum_t ln acc - T*SHIFT
    losses = np.zeros(NCORES * NL, np.float64)
    for c in range(NCORES):
        resv = np.asarray(out.results[c]["res"]).astype(np.float64)
        accv = np.asarray(out.results[c]["accd"]).astype(np.float64)
        if not (np.isfinite(accv).all() and (accv > 0).all()):
            return None
        lacc = np.log(accv).reshape(NL, TG, NG_).sum(axis=(1, 2))  # per n
        for n in range(NL):
            gn = c * NL + n
            ar = resv[n, PAD:]            # alpha_{TH-1}
            br = resv[NL + n, PAD:]       # beta_{TH}, reversed s order
            if use_masks:
                a, b = ar, br
            else:
                # de-interleave [labels | blanks] back to natural order
                a = np.empty(S_)
                a[1::2] = ar[0:L_]
                a[0::2] = ar[L_:S_]
                b = np.empty(S_)
                b[1::2] = br[0:L_]
                b[0::2] = br[L_:S_]
            mb = m_bwd[gn]
            be = b.copy()
            be[1:] += b[:-1]
            be[2:] += np.where(mb[2:], b[:-2], 0.0)
            v = float((a[::-1] * be).sum())
            if not (np.isfinite(v) and v > 0.0):
                return None
            ll = np.log(v) - lacc[n] - T_ * SHIFT2
            losses[gn] = -ll / L_
    return np.float32(losses.mean())


# ----------------------------------------------------------------------------
# entry point
# ----------------------------------------------------------------------------

def kernel(preds, targets, pred_lengths, target_lengths):
    preds = np.asarray(preds, np.float32)
    targets = np.asarray(targets, np.int32)
    pred_lengths = np.asarray(pred_lengths, np.int32)
    target_lengths = np.asarray(target_lengths, np.int32)
    t2d = targets.reshape(N, L)

    fast_ok = (
        preds.shape == (T, N, C)
        and targets.shape == (N * L,)
        and np.all(pred_lengths == T)
        and np.all(target_lengths == L)
        and np.all(targets >= 1)
        and np.all(targets < C)
        and np.isfinite(preds).all()
        and np.abs(preds).max() < 8.0
    )
    if fast_ok:
        r = _run_device(preds, t2d, (T, C, L))
        if r is not None:
            return r
    return _ref_numpy(preds, t2d, pred_lengths, target_lengths)
